# revision 16
# baseline (speedup 1.0000x reference)
"""Trainium2 Bass kernel for nn_ContrastiveDistortion (symmetric pairwise-KL InfoNCE loss).

Math: with IS_SYMMETRIC=True the logdet terms cancel and
  logits_sym[a,b] = D/2 - U[a,b]/4,
  U[a,b] = <inv_a, q_b> + <q_a, inv_b> + <mi_a, mud_b> + <mud_a, mi_b> + c_a + c_b
with q = var + mu^2, mi = mu/var, mud = -2*mu, c = sum_d mu^2/var. All five
derived planes are pure per-item functions, so the HOST precomputes them in
float64 and DMAs them in ([128,4096] feature-major, column-ROTATED by 512*k per
core so the program is SPMD-identical: the core's own 512-row block is local
columns 0..511, the positive pairs local columns 2048..2559). c_a is a row
constant (cancels in log-softmax); c_b rides in as a partition-broadcast plane.

The PE runs 4 K=128 chunks per [128,512] output group into [128,1024] PSUM
slabs (4 in flight = all 8 banks). One tensor_tensor add per slab drains PSUM
to fp16 `us` tiles while adding c_b (the diagonal mask is pre-folded into a
+60000 stripe of the cb plane); drains are split DVE/Pool. A 2x-rate fp16 DVE
reduce finds each half-row min, ACT exponentiates with scale=-SCL/bias=SCL*min
and row-sum accum, Pool extracts the positive logits. Per-row partials
(SCL*min, expsum, pos value) go to the host for the final logsumexp/mean in
float64.
"""

import sys
from contextlib import ExitStack

import numpy as np

sys.path.insert(0, "/opt/trn_rl_repo")

import concourse.bass as bass
import concourse.bacc as bacc_mod
import concourse.mybir as mybir
from concourse.bass_utils import run_bass_kernel_spmd
from concourse.tile import TileContext

F32 = mybir.dt.float32
F32R = mybir.dt.float32r
F16 = mybir.dt.float16
BF16 = mybir.dt.bfloat16
I32 = mybir.dt.int32
AF = mybir.ActivationFunctionType
ALU = mybir.AluOpType
AX = mybir.AxisListType

P = 128          # partitions / feature dim D
NB = 4096        # N = 2B rows
NC = 8           # cores
RB = NB // NC    # 512 rows per core
NM = RB // P     # 4 m-chunks of 128 rows
HALF = NB // 2   # 2048 columns per softmax half
SLAB = 1024      # PSUM slab width (2 banks)
TEMPERATURE = 0.1
WEIGHT = 5.0
SCL = 1.0 / (4.0 * TEMPERATURE)  # 2.5: l = -SCL*U + const_row
MASKC = 60000.0  # diagonal mask offset (stays finite in fp16)

# slabs whose PSUM drain runs on the Pool engine (s==1 slabs of these hm
# tiles); set () if Pool cannot access PSUM on this runtime
POOL_DRAIN_HM = ()


def _build_nc():
    nc = bacc_mod.Bacc(None, target_bir_lowering=False, name="contrastive_distortion")
    inv_d = nc.declare_dram_parameter("invT", [P, NB], BF16, isOutput=False)
    q_d = nc.declare_dram_parameter("qT", [P, NB], BF16, isOutput=False)
    mi_d = nc.declare_dram_parameter("miT", [P, NB], BF16, isOutput=False)
    mud_d = nc.declare_dram_parameter("mudT", [P, NB], BF16, isOutput=False)
    cb_d = nc.declare_dram_parameter("cbB", [P, NB], F32, isOutput=False)
    cbK_d = nc.declare_dram_parameter("cbKB", [P, NB], F32R, isOutput=False)
    out_d = nc.declare_dram_parameter("out", [P, 20], F32, isOutput=True)

    with TileContext(nc) as tc, ExitStack() as ctx:
        big = ctx.enter_context(tc.tile_pool(name="big", bufs=1))
        sm = ctx.enter_context(tc.tile_pool(name="sm", bufs=1))
        scr = ctx.enter_context(tc.tile_pool(name="scr", bufs=2))
        pp = ctx.enter_context(tc.tile_pool(name="pp", bufs=4, space="PSUM"))

        inv = big.tile([P, NB], BF16)
        q = big.tile([P, NB], BF16)
        mi = big.tile([P, NB], BF16)
        mud = big.tile([P, NB], BF16)
        cb = big.tile([P, NB], F32)    # h0 cols only
        cbK = big.tile([P, NB], F32R)  # c_b/128, h1 cols only (5th chunk rhs)
        ones = big.tile([P, P], F32R)
        ones_f = sm.tile([P, P], F32)
        oneh = big.tile([P, RB * NM], F32)  # [128, 2048] stripe one-hots per m
        cbm = big.tile([P, RB * NM], F32)   # cb[0:512] + MASKC at the diagonal

        ioti = sm.tile([P, RB], I32)
        mnf = sm.tile([P, 8], F32)
        outpack = sm.tile([P, 20], F32)  # 0:8 SCL*min, 8:16 expsum, 16:20 upos
        e2k = sm.tile([P, HALF], F32)
        junk = sm.tile([P, RB], F32)

        nc.vector.memset(ones_f, 1.0)
        nc.vector.tensor_copy(out=ones, in_=ones_f)
        # ioti[p, c] = c - p ; onehot_m[p, c] = (c - p == 128*m)
        nc.gpsimd.iota(ioti, pattern=[[1, RB]], base=0, channel_multiplier=-1)
        for m in range(NM):
            nc.vector.tensor_single_scalar(
                out=oneh[:, RB * m:RB * (m + 1)], in_=ioti, scalar=P * m,
                op=ALU.is_equal)

        # input DMAs in 1024-col quarters: bf16 planes on the sync HWDGE
        # queue, the f32 cb plane on the gpsimd SWDGE queue
        for qd in range(4):
            sl = slice(SLAB * qd, SLAB * (qd + 1))
            nc.sync.dma_start(out=inv[:, sl], in_=inv_d[:, sl])
            nc.sync.dma_start(out=q[:, sl], in_=q_d[:, sl])
            nc.sync.dma_start(out=mi[:, sl], in_=mi_d[:, sl])
            nc.sync.dma_start(out=mud[:, sl], in_=mud_d[:, sl])
            if qd < 2:  # cb only biases the h0 drains
                nc.gpsimd.dma_start(out=cb[:, sl], in_=cb_d[:, sl])
            else:       # h1 gets c_b through the 5th matmul chunk
                nc.gpsimd.dma_start(out=cbK[:, sl], in_=cbK_d[:, sl])

        # cbm_m = cb[0:512] + MASKC*onehot_m (mask pre-folded into the drain)
        for m in range(NM):
            nc.vector.scalar_tensor_tensor(
                out=cbm[:, RB * m:RB * (m + 1)],
                in0=oneh[:, RB * m:RB * (m + 1)], scalar=MASKC,
                in1=cb[:, 0:RB], op0=ALU.mult, op1=ALU.add)

        chunks = [(inv, q), (q, inv), (mi, mud), (mud, mi)]
        slab_release = []  # SBUF AP whose write releases that psum slot
        si = 0
        for h in range(2):
            for m in range(NM):
                hm = 4 * h + m
                us = scr.tile([P, HALF], F32, name=f"us{hm}", tag="us", bufs=2)
                for s in range(2):
                    u = pp.tile([P, SLAB], F32, name=f"u{h}{m}{s}", tag="ps")
                    # Matmul instructions can carry only ONE sync wait in
                    # walrus codegen. A psum-slot-reusing group head needs two
                    # deps: WAW vs the old matmuls (PE sem) and WAR vs the old
                    # slab's drain read-out. This orphan ldweights (no PSUM
                    # write -> no PE wait) absorbs the drain dep via the token
                    # written right after that read-out, leaving the real
                    # matmuls with just the PE-completion wait.
                    if si >= 4:
                        nc.tensor.ldweights(slab_release[si - 4])
                    for j in range(2):
                        osl = slice(512 * j, 512 * (j + 1))
                        gsl = slice(HALF * h + SLAB * s + 512 * j,
                                    HALF * h + SLAB * s + 512 * (j + 1))
                        for ci, (lp, rp) in enumerate(chunks):
                            nc.tensor.matmul(
                                u[:, osl], lhsT=lp[:, P * m:P * (m + 1)],
                                rhs=rp[:, gsl],
                                start=(ci == 0), stop=(ci == 3 and h == 0))
                        if h == 1:
                            # f32r rank-128 chunk adds c_b = 128*(c_b/128)
                            nc.tensor.matmul(
                                u[:, osl], lhsT=ones, rhs=cbK[:, gsl],
                                start=False, stop=True)
                    ssl = slice(SLAB * s, SLAB * (s + 1))
                    csl = slice(HALF * h + SLAB * s, HALF * h + SLAB * (s + 1))
                    if h == 0 and s == 0:
                        # drain with the masked cb stripe on the block columns
                        nc.vector.tensor_add(us[:, 512:SLAB], u[:, 512:SLAB],
                                             cb[:, 512:SLAB])
                        nc.vector.tensor_add(us[:, 0:512], u[:, 0:512],
                                             cbm[:, RB * m:RB * (m + 1)])
                        rel = us.bitcast(BF16)[0:1, 0:2]
                    elif h == 1:
                        # c_b already accumulated on the PE; plain ACT drain
                        nc.scalar.activation(out=us[:, ssl], in_=u,
                                             func=AF.Copy)
                        rel = us.bitcast(BF16)[0:1, 2 * SLAB * s:2 * SLAB * s + 2]
                    else:
                        nc.vector.tensor_add(us[:, ssl], u, cb[:, csl])
                        rel = us.bitcast(BF16)[0:1, 2 * SLAB * s:2 * SLAB * s + 2]
                    slab_release.append(rel)
                    si += 1
                nc.vector.tensor_reduce(mnf[:, hm:hm + 1], us, axis=AX.X,
                                        op=ALU.min)
                nc.vector.tensor_scalar_mul(outpack[:, hm:hm + 1],
                                            mnf[:, hm:hm + 1], SCL)
                # exp(SCL*min - SCL*us) with row-sum accumulated into outpack
                nc.scalar.activation(
                    out=e2k, in_=us, func=AF.Exp, bias=outpack[:, hm:hm + 1],
                    scale=-SCL, accum_out=outpack[:, 8 + hm:9 + hm])
                if h == 1:
                    # positive logits live at local cols 2048+128m+p
                    s512 = scr.tile([P, RB], F32, name=f"s512{m}", tag="s512",
                                    bufs=2)
                    nc.gpsimd.tensor_mul(s512, us[:, 0:RB],
                                         oneh[:, RB * m:RB * (m + 1)])
                    nc.scalar.activation(
                        out=junk, in_=s512, func=AF.Identity,
                        accum_out=outpack[:, 16 + m:17 + m])
        nc.sync.dma_start(out=out_d[:, :], in_=outpack)

    return nc


_NC_CACHE = None


def _get_nc():
    global _NC_CACHE
    if _NC_CACHE is None:
        nc = _build_nc()
        nc.finalize()  # runs Bacc.compile(): wait legalization for TRN2
        _NC_CACHE = nc
    return _NC_CACHE


def _host_planes(mu_x, sigma_x, mu_p, sigma_p):
    mus = np.concatenate([np.asarray(mu_x, np.float64),
                          np.asarray(mu_p, np.float64)], 0)
    sigmas = np.concatenate([np.asarray(sigma_x, np.float64),
                             np.asarray(sigma_p, np.float64)], 0)
    var = sigmas * sigmas
    inv = 1.0 / var
    planes = {
        "invT": inv.T,
        "qT": (var + mus * mus).T,
        "miT": (mus * inv).T,
        "mudT": (-2.0 * mus).T,
    }
    import ml_dtypes
    planes = {k: np.ascontiguousarray(v.astype(np.float32),
                                      ml_dtypes.bfloat16)
              for k, v in planes.items()}
    c = (mus * mus * inv).sum(1).astype(np.float32)
    return planes, c


def run_sharded(mu_x, sigma_x, mu_p, sigma_p, trace=False):
    planes, c = _host_planes(mu_x, sigma_x, mu_p, sigma_p)
    in_maps = []
    for k in range(NC):
        m = {name: np.ascontiguousarray(np.roll(v, -RB * k, axis=1))
             for name, v in planes.items()}
        ck = np.roll(c, -RB * k)
        m["cbB"] = np.ascontiguousarray(
            np.broadcast_to(ck[None, :], (P, NB)))
        m["cbKB"] = np.ascontiguousarray(
            np.broadcast_to((ck / P)[None, :], (P, NB)))
        in_maps.append(m)
    kwargs = {}
    if trace:
        kwargs = dict(trace=True, trace_cores=[0])
    br = run_bass_kernel_spmd(_get_nc(), in_maps, core_ids=list(range(NC)),
                              **kwargs)
    outs = np.stack([np.asarray(r["out"], np.float64) for r in br.results])
    smn = outs[:, :, 0:8]    # per-(h,m) SCL*min of us
    es = outs[:, :, 8:16]    # per-(h,m) sum exp(SCL*min - SCL*us)
    up = outs[:, :, 16:20]   # us at the positive column
    L = -smn + np.log(es)    # per-half log sum exp(-SCL*us)
    LSE = np.logaddexp(L[:, :, 0:4], L[:, :, 4:8])
    rl = LSE + SCL * up      # row loss (row consts cancel)
    n_classes = NB - 1
    to_mult = (n_classes - 1.0 / WEIGHT) / (n_classes - 1)
    to_add = -np.log(np.float32(to_mult))
    loss = np.float32(rl.sum() / NB - to_add)
    return loss, br


def kernel(z_hat, mu_x, sigma_x, mu_p, sigma_p):
    loss, _ = run_sharded(mu_x, sigma_x, mu_p, sigma_p)
    return np.asarray(loss, np.float32)


# revision 17
# speedup vs baseline: 1.0313x; 1.0313x over previous
"""Trainium2 Bass kernel for nn_ContrastiveDistortion (symmetric pairwise-KL InfoNCE loss).

Math: with IS_SYMMETRIC=True the logdet terms cancel and
  logits_sym[a,b] = D/2 - U[a,b]/4,
  U[a,b] = <inv_a, q_b> + <q_a, inv_b> + <mi_a, mud_b> + <mud_a, mi_b> + c_a + c_b
with q = var + mu^2, mi = mu/var, mud = -2*mu, c = sum_d mu^2/var. All five
derived planes are pure per-item functions, so the HOST precomputes them in
float64 and DMAs them in ([128,4096] feature-major, column-ROTATED by 512*k per
core so the program is SPMD-identical: the core's own 512-row block is local
columns 0..511, the positive pairs local columns 2048..2559). c_a is a row
constant (cancels in log-softmax); c_b rides in as a partition-broadcast plane.

The PE runs 4 K=128 chunks per [128,512] output group into [128,1024] PSUM
slabs (4 in flight = all 8 banks). One tensor_tensor add per slab drains PSUM
to fp16 `us` tiles while adding c_b (the diagonal mask is pre-folded into a
+60000 stripe of the cb plane); drains are split DVE/Pool. A 2x-rate fp16 DVE
reduce finds each half-row min, ACT exponentiates with scale=-SCL/bias=SCL*min
and row-sum accum, Pool extracts the positive logits. Per-row partials
(SCL*min, expsum, pos value) go to the host for the final logsumexp/mean in
float64.
"""

import sys
from contextlib import ExitStack

import numpy as np

sys.path.insert(0, "/opt/trn_rl_repo")

import concourse.bass as bass
import concourse.bacc as bacc_mod
import concourse.mybir as mybir
from concourse.bass_utils import run_bass_kernel_spmd
from concourse.tile import TileContext

F32 = mybir.dt.float32
F32R = mybir.dt.float32r
F16 = mybir.dt.float16
BF16 = mybir.dt.bfloat16
I32 = mybir.dt.int32
AF = mybir.ActivationFunctionType
ALU = mybir.AluOpType
AX = mybir.AxisListType

P = 128          # partitions / feature dim D
NB = 4096        # N = 2B rows
NC = 8           # cores
RB = NB // NC    # 512 rows per core
NM = RB // P     # 4 m-chunks of 128 rows
HALF = NB // 2   # 2048 columns per softmax half
SLAB = 1024      # PSUM slab width (2 banks)
TEMPERATURE = 0.1
WEIGHT = 5.0
SCL = 1.0 / (4.0 * TEMPERATURE)  # 2.5: l = -SCL*U + const_row
MASKC = 60000.0  # diagonal mask offset (stays finite in fp16)

# slabs whose PSUM drain runs on the Pool engine (s==1 slabs of these hm
# tiles); set () if Pool cannot access PSUM on this runtime
POOL_DRAIN_HM = ()


def _build_nc():
    nc = bacc_mod.Bacc(None, target_bir_lowering=False, name="contrastive_distortion")
    inv_d = nc.declare_dram_parameter("invT", [P, NB], BF16, isOutput=False)
    q_d = nc.declare_dram_parameter("qT", [P, NB], BF16, isOutput=False)
    mi_d = nc.declare_dram_parameter("miT", [P, NB], BF16, isOutput=False)
    mud_d = nc.declare_dram_parameter("mudT", [P, NB], BF16, isOutput=False)
    cb_d = nc.declare_dram_parameter("cbB", [P, NB], F32, isOutput=False)
    cbK_d = nc.declare_dram_parameter("cbKB", [P, NB], F32R, isOutput=False)
    out_d = nc.declare_dram_parameter("out", [P, 20], F32, isOutput=True)

    with TileContext(nc) as tc, ExitStack() as ctx:
        big = ctx.enter_context(tc.tile_pool(name="big", bufs=1))
        sm = ctx.enter_context(tc.tile_pool(name="sm", bufs=1))
        scr = ctx.enter_context(tc.tile_pool(name="scr", bufs=2))
        pp = ctx.enter_context(tc.tile_pool(name="pp", bufs=4, space="PSUM"))

        inv = big.tile([P, NB], BF16)
        q = big.tile([P, NB], BF16)
        mi = big.tile([P, NB], BF16)
        mud = big.tile([P, NB], BF16)
        cb = big.tile([P, NB], F32)    # h0 cols only
        cbK = big.tile([P, NB], F32R)  # c_b/128, h1 cols only (5th chunk rhs)
        ones = big.tile([P, P], F32R)
        ones_f = sm.tile([P, P], F32)
        oneh = big.tile([P, RB * NM], F32)  # [128, 2048] stripe one-hots per m
        cbm = big.tile([P, RB * NM], F32)   # cb[0:512] + MASKC at the diagonal

        ioti = sm.tile([P, RB], I32)
        mnf = sm.tile([P, 8], F32)
        bias8 = sm.tile([P, 8], F32)   # SCL*min per (h,m)
        es8 = sm.tile([P, 8], F32)     # expsum per (h,m)
        upos4 = sm.tile([P, 4], F32)   # us at the positive column
        e2k = sm.tile([P, HALF], F32)
        junk = sm.tile([P, RB], F32)

        nc.vector.memset(ones_f, 1.0)
        nc.vector.tensor_copy(out=ones, in_=ones_f)
        # ioti[p, c] = c - p ; onehot_m[p, c] = (c - p == 128*m)
        nc.gpsimd.iota(ioti, pattern=[[1, RB]], base=0, channel_multiplier=-1)
        for m in range(NM):
            nc.vector.tensor_single_scalar(
                out=oneh[:, RB * m:RB * (m + 1)], in_=ioti, scalar=P * m,
                op=ALU.is_equal)

        # input DMAs in 1024-col quarters: bf16 planes on the sync HWDGE
        # queue, the f32 cb plane on the gpsimd SWDGE queue
        for qd in range(4):
            sl = slice(SLAB * qd, SLAB * (qd + 1))
            nc.sync.dma_start(out=inv[:, sl], in_=inv_d[:, sl])
            nc.sync.dma_start(out=q[:, sl], in_=q_d[:, sl])
            nc.sync.dma_start(out=mi[:, sl], in_=mi_d[:, sl])
            nc.sync.dma_start(out=mud[:, sl], in_=mud_d[:, sl])
            if qd < 2:  # cb only biases the h0 drains
                nc.gpsimd.dma_start(out=cb[:, sl], in_=cb_d[:, sl])
            else:       # h1 gets c_b through the 5th matmul chunk
                nc.gpsimd.dma_start(out=cbK[:, sl], in_=cbK_d[:, sl])

        # cbm_m = cb[0:512] + MASKC*onehot_m (mask pre-folded into the drain)
        for m in range(NM):
            nc.vector.scalar_tensor_tensor(
                out=cbm[:, RB * m:RB * (m + 1)],
                in0=oneh[:, RB * m:RB * (m + 1)], scalar=MASKC,
                in1=cb[:, 0:RB], op0=ALU.mult, op1=ALU.add)

        chunks = [(inv, q), (q, inv), (mi, mud), (mud, mi)]
        slab_release = []  # SBUF AP whose write releases that psum slot
        si = 0
        for h in range(2):
            for m in range(NM):
                hm = 4 * h + m
                us = scr.tile([P, HALF], F32, name=f"us{hm}", tag="us", bufs=3)
                for s in range(2):
                    u = pp.tile([P, SLAB], F32, name=f"u{h}{m}{s}", tag="ps")
                    # Matmul instructions can carry only ONE sync wait in
                    # walrus codegen. A psum-slot-reusing group head needs two
                    # deps: WAW vs the old matmuls (PE sem) and WAR vs the old
                    # slab's drain read-out. This orphan ldweights (no PSUM
                    # write -> no PE wait) absorbs the drain dep via the token
                    # written right after that read-out, leaving the real
                    # matmuls with just the PE-completion wait.
                    if si >= 4:
                        nc.tensor.ldweights(slab_release[si - 4])
                    for j in range(2):
                        osl = slice(512 * j, 512 * (j + 1))
                        gsl = slice(HALF * h + SLAB * s + 512 * j,
                                    HALF * h + SLAB * s + 512 * (j + 1))
                        for ci, (lp, rp) in enumerate(chunks):
                            nc.tensor.matmul(
                                u[:, osl], lhsT=lp[:, P * m:P * (m + 1)],
                                rhs=rp[:, gsl],
                                start=(ci == 0), stop=(ci == 3 and h == 0))
                        if h == 1:
                            # f32r rank-128 chunk adds c_b = 128*(c_b/128)
                            nc.tensor.matmul(
                                u[:, osl], lhsT=ones, rhs=cbK[:, gsl],
                                start=False, stop=True)
                    ssl = slice(SLAB * s, SLAB * (s + 1))
                    csl = slice(HALF * h + SLAB * s, HALF * h + SLAB * (s + 1))
                    if h == 0 and s == 0:
                        # drain with the masked cb stripe on the block columns
                        nc.vector.tensor_add(us[:, 512:SLAB], u[:, 512:SLAB],
                                             cb[:, 512:SLAB])
                        nc.vector.tensor_add(us[:, 0:512], u[:, 0:512],
                                             cbm[:, RB * m:RB * (m + 1)])
                        rel = us.bitcast(BF16)[0:1, 0:2]
                    elif h == 1:
                        # c_b already accumulated on the PE; plain ACT drain
                        nc.scalar.activation(out=us[:, ssl], in_=u,
                                             func=AF.Copy)
                        rel = us.bitcast(BF16)[0:1, 2 * SLAB * s:2 * SLAB * s + 2]
                    else:
                        nc.vector.tensor_add(us[:, ssl], u, cb[:, csl])
                        rel = us.bitcast(BF16)[0:1, 2 * SLAB * s:2 * SLAB * s + 2]
                    slab_release.append(rel)
                    si += 1
                nc.vector.tensor_reduce(mnf[:, hm:hm + 1], us, axis=AX.X,
                                        op=ALU.min)
                nc.vector.tensor_scalar_mul(bias8[:, hm:hm + 1],
                                            mnf[:, hm:hm + 1], SCL)
                # exp(SCL*min - SCL*us) with row-sum accumulated into outpack
                nc.scalar.activation(
                    out=e2k, in_=us, func=AF.Exp, bias=bias8[:, hm:hm + 1],
                    scale=-SCL, accum_out=es8[:, hm:hm + 1])
                if h == 1:
                    # positive logits live at local cols 2048+128m+p
                    s512 = scr.tile([P, RB], F32, name=f"s512{m}", tag="s512",
                                    bufs=2)
                    nc.gpsimd.tensor_mul(s512, us[:, 0:RB],
                                         oneh[:, RB * m:RB * (m + 1)])
                    nc.scalar.activation(
                        out=junk, in_=s512, func=AF.Identity,
                        accum_out=upos4[:, m:m + 1])
        nc.sync.dma_start(out=out_d[:, 0:8], in_=bias8)
        nc.sync.dma_start(out=out_d[:, 8:16], in_=es8)
        nc.sync.dma_start(out=out_d[:, 16:20], in_=upos4)

    return nc


_NC_CACHE = None


def _get_nc():
    global _NC_CACHE
    if _NC_CACHE is None:
        nc = _build_nc()
        nc.finalize()  # runs Bacc.compile(): wait legalization for TRN2
        _NC_CACHE = nc
    return _NC_CACHE


def _host_planes(mu_x, sigma_x, mu_p, sigma_p):
    mus = np.concatenate([np.asarray(mu_x, np.float64),
                          np.asarray(mu_p, np.float64)], 0)
    sigmas = np.concatenate([np.asarray(sigma_x, np.float64),
                             np.asarray(sigma_p, np.float64)], 0)
    var = sigmas * sigmas
    inv = 1.0 / var
    planes = {
        "invT": inv.T,
        "qT": (var + mus * mus).T,
        "miT": (mus * inv).T,
        "mudT": (-2.0 * mus).T,
    }
    import ml_dtypes
    planes = {k: np.ascontiguousarray(v.astype(np.float32),
                                      ml_dtypes.bfloat16)
              for k, v in planes.items()}
    c = (mus * mus * inv).sum(1).astype(np.float32)
    return planes, c


def run_sharded(mu_x, sigma_x, mu_p, sigma_p, trace=False):
    planes, c = _host_planes(mu_x, sigma_x, mu_p, sigma_p)
    in_maps = []
    for k in range(NC):
        m = {name: np.ascontiguousarray(np.roll(v, -RB * k, axis=1))
             for name, v in planes.items()}
        ck = np.roll(c, -RB * k)
        m["cbB"] = np.ascontiguousarray(
            np.broadcast_to(ck[None, :], (P, NB)))
        m["cbKB"] = np.ascontiguousarray(
            np.broadcast_to((ck / P)[None, :], (P, NB)))
        in_maps.append(m)
    kwargs = {}
    if trace:
        kwargs = dict(trace=True, trace_cores=[0])
    br = run_bass_kernel_spmd(_get_nc(), in_maps, core_ids=list(range(NC)),
                              **kwargs)
    outs = np.stack([np.asarray(r["out"], np.float64) for r in br.results])
    smn = outs[:, :, 0:8]    # per-(h,m) SCL*min of us
    es = outs[:, :, 8:16]    # per-(h,m) sum exp(SCL*min - SCL*us)
    up = outs[:, :, 16:20]   # us at the positive column
    L = -smn + np.log(es)    # per-half log sum exp(-SCL*us)
    LSE = np.logaddexp(L[:, :, 0:4], L[:, :, 4:8])
    rl = LSE + SCL * up      # row loss (row consts cancel)
    n_classes = NB - 1
    to_mult = (n_classes - 1.0 / WEIGHT) / (n_classes - 1)
    to_add = -np.log(np.float32(to_mult))
    loss = np.float32(rl.sum() / NB - to_add)
    return loss, br


def kernel(z_hat, mu_x, sigma_x, mu_p, sigma_p):
    loss, _ = run_sharded(mu_x, sigma_x, mu_p, sigma_p)
    return np.asarray(loss, np.float32)


# revision 18
# speedup vs baseline: 1.1254x; 1.0913x over previous
"""Trainium2 Bass kernel for nn_ContrastiveDistortion (symmetric pairwise-KL InfoNCE loss).

Math: with IS_SYMMETRIC=True the logdet terms cancel and
  logits_sym[a,b] = D/2 - U[a,b]/4,
  U[a,b] = <inv_a, q_b> + <q_a, inv_b> + <mi_a, mud_b> + <mud_a, mi_b> + c_a + c_b
with q = var + mu^2, mi = mu/var, mud = -2*mu, c = sum_d mu^2/var. All five
derived planes are pure per-item functions, so the HOST precomputes them in
float64 and DMAs them in ([128,4096] feature-major, column-ROTATED by 512*k per
core so the program is SPMD-identical: the core's own 512-row block is local
columns 0..511, the positive pairs local columns 2048..2559). c_a is a row
constant (cancels in log-softmax); c_b rides in as a partition-broadcast plane.

The PE runs 4 K=128 chunks per [128,512] output group into [128,1024] PSUM
slabs (4 in flight = all 8 banks). One tensor_tensor add per slab drains PSUM
to fp16 `us` tiles while adding c_b (the diagonal mask is pre-folded into a
+60000 stripe of the cb plane); drains are split DVE/Pool. A 2x-rate fp16 DVE
reduce finds each half-row min, ACT exponentiates with scale=-SCL/bias=SCL*min
and row-sum accum, Pool extracts the positive logits. Per-row partials
(SCL*min, expsum, pos value) go to the host for the final logsumexp/mean in
float64.
"""

import sys
from contextlib import ExitStack

import numpy as np

sys.path.insert(0, "/opt/trn_rl_repo")

import concourse.bass as bass
import concourse.bacc as bacc_mod
import concourse.mybir as mybir
from concourse.bass_utils import run_bass_kernel_spmd
from concourse.tile import TileContext

F32 = mybir.dt.float32
F32R = mybir.dt.float32r
F16 = mybir.dt.float16
BF16 = mybir.dt.bfloat16
I32 = mybir.dt.int32
AF = mybir.ActivationFunctionType
ALU = mybir.AluOpType
AX = mybir.AxisListType

P = 128          # partitions / feature dim D
NB = 4096        # N = 2B rows
NC = 8           # cores
RB = NB // NC    # 512 rows per core
NM = RB // P     # 4 m-chunks of 128 rows
HALF = NB // 2   # 2048 columns per softmax half
SLAB = 1024      # PSUM slab width (2 banks)
TEMPERATURE = 0.1
WEIGHT = 5.0
SCL = 1.0 / (4.0 * TEMPERATURE)  # 2.5: l = -SCL*U + const_row
MASKC = 60000.0  # diagonal mask offset (stays finite in fp16)

# slabs whose PSUM drain runs on the Pool engine (s==1 slabs of these hm
# tiles); set () if Pool cannot access PSUM on this runtime
POOL_DRAIN_HM = ()


def _build_nc():
    nc = bacc_mod.Bacc(None, target_bir_lowering=False, name="contrastive_distortion")
    inv_d = nc.declare_dram_parameter("invT", [P, NB], BF16, isOutput=False)
    q_d = nc.declare_dram_parameter("qT", [P, NB], BF16, isOutput=False)
    mi_d = nc.declare_dram_parameter("miT", [P, NB], BF16, isOutput=False)
    mud_d = nc.declare_dram_parameter("mudT", [P, NB], BF16, isOutput=False)
    cb_d = nc.declare_dram_parameter("cbB", [P, NB], F32, isOutput=False)
    cbK_d = nc.declare_dram_parameter("cbKB", [P, NB], F32R, isOutput=False)
    out_d = nc.declare_dram_parameter("out", [P, 20], F32, isOutput=True)

    with TileContext(nc) as tc, ExitStack() as ctx:
        big = ctx.enter_context(tc.tile_pool(name="big", bufs=1))
        sm = ctx.enter_context(tc.tile_pool(name="sm", bufs=1))
        scr = ctx.enter_context(tc.tile_pool(name="scr", bufs=2))
        pp = ctx.enter_context(tc.tile_pool(name="pp", bufs=4, space="PSUM"))

        inv = big.tile([P, NB], BF16)
        q = big.tile([P, NB], BF16)
        mi = big.tile([P, NB], BF16)
        mud = big.tile([P, NB], BF16)
        cb = big.tile([P, NB], F32)    # h0 cols only
        cbK = big.tile([P, NB], F32R)  # c_b/128, h1 cols only (5th chunk rhs)
        ones = big.tile([P, P], F32R)
        ones_f = sm.tile([P, P], F32)
        oneh = big.tile([P, RB * NM], F32)  # [128, 2048] stripe one-hots per m
        cbm = big.tile([P, RB * NM], F32)   # cb[0:512] + MASKC at the diagonal

        ioti = sm.tile([P, RB], I32)
        mnf = sm.tile([P, 8], F32)
        bias8 = sm.tile([P, 8], F32)   # SCL*min per (h,m)
        es8 = sm.tile([P, 8], F32)     # expsum per (h,m)
        upos4 = sm.tile([P, 4], F32)   # us at the positive column
        e2k = sm.tile([P, HALF], F32)
        junk = sm.tile([P, RB], F32)

        nc.vector.memset(ones_f, 1.0)
        nc.vector.tensor_copy(out=ones, in_=ones_f)
        # ioti[p, c] = c - p ; onehot_m[p, c] = (c - p == 128*m)
        nc.gpsimd.iota(ioti, pattern=[[1, RB]], base=0, channel_multiplier=-1)
        for m in range(NM):
            nc.vector.tensor_single_scalar(
                out=oneh[:, RB * m:RB * (m + 1)], in_=ioti, scalar=P * m,
                op=ALU.is_equal)

        # input DMAs in 1024-col quarters on ONE queue, ordered to match
        # consumption (the DMA engines serialize transfers): planes of
        # quarter qd, then the cb/cbK quarter the drains/5th-chunk need next
        for qd in range(4):
            sl = slice(SLAB * qd, SLAB * (qd + 1))
            nc.sync.dma_start(out=inv[:, sl], in_=inv_d[:, sl])
            nc.sync.dma_start(out=q[:, sl], in_=q_d[:, sl])
            nc.sync.dma_start(out=mi[:, sl], in_=mi_d[:, sl])
            nc.sync.dma_start(out=mud[:, sl], in_=mud_d[:, sl])
            if qd < 2:  # cb only biases the h0 drains
                nc.sync.dma_start(out=cb[:, sl], in_=cb_d[:, sl])
            else:       # h1 gets c_b through the 5th matmul chunk
                nc.sync.dma_start(out=cbK[:, sl], in_=cbK_d[:, sl])

        # cbm_m = cb[0:512] + MASKC*onehot_m (mask pre-folded into the drain)
        for m in range(NM):
            nc.vector.scalar_tensor_tensor(
                out=cbm[:, RB * m:RB * (m + 1)],
                in0=oneh[:, RB * m:RB * (m + 1)], scalar=MASKC,
                in1=cb[:, 0:RB], op0=ALU.mult, op1=ALU.add)

        chunks = [(inv, q), (q, inv), (mi, mud), (mud, mi)]
        slab_release = []  # SBUF AP whose write releases that psum slot
        si = 0
        for h in range(2):
            for m in range(NM):
                hm = 4 * h + m
                us = scr.tile([P, HALF], F32, name=f"us{hm}", tag="us", bufs=3)
                for s in range(2):
                    u = pp.tile([P, SLAB], F32, name=f"u{h}{m}{s}", tag="ps")
                    # Matmul instructions can carry only ONE sync wait in
                    # walrus codegen. A psum-slot-reusing group head needs two
                    # deps: WAW vs the old matmuls (PE sem) and WAR vs the old
                    # slab's drain read-out. This orphan ldweights (no PSUM
                    # write -> no PE wait) absorbs the drain dep via the token
                    # written right after that read-out, leaving the real
                    # matmuls with just the PE-completion wait.
                    if si >= 4:
                        nc.tensor.ldweights(slab_release[si - 4])
                    for j in range(2):
                        osl = slice(512 * j, 512 * (j + 1))
                        gsl = slice(HALF * h + SLAB * s + 512 * j,
                                    HALF * h + SLAB * s + 512 * (j + 1))
                        for ci, (lp, rp) in enumerate(chunks):
                            nc.tensor.matmul(
                                u[:, osl], lhsT=lp[:, P * m:P * (m + 1)],
                                rhs=rp[:, gsl],
                                start=(ci == 0), stop=(ci == 3 and h == 0))
                        if h == 1:
                            # f32r rank-128 chunk adds c_b = 128*(c_b/128)
                            nc.tensor.matmul(
                                u[:, osl], lhsT=ones, rhs=cbK[:, gsl],
                                start=False, stop=True)
                    ssl = slice(SLAB * s, SLAB * (s + 1))
                    csl = slice(HALF * h + SLAB * s, HALF * h + SLAB * (s + 1))
                    if h == 0 and s == 0:
                        # drain with the masked cb stripe on the block columns
                        nc.vector.tensor_add(us[:, 512:SLAB], u[:, 512:SLAB],
                                             cb[:, 512:SLAB])
                        nc.vector.tensor_add(us[:, 0:512], u[:, 0:512],
                                             cbm[:, RB * m:RB * (m + 1)])
                        rel = us.bitcast(BF16)[0:1, 0:2]
                    elif h == 1 and s == 1:
                        # c_b already accumulated on the PE; plain ACT drain
                        nc.scalar.activation(out=us[:, ssl], in_=u,
                                             func=AF.Copy)
                        rel = us.bitcast(BF16)[0:1, 2 * SLAB * s:2 * SLAB * s + 2]
                    elif h == 1:
                        nc.vector.tensor_copy(out=us[:, ssl], in_=u)
                        rel = us.bitcast(BF16)[0:1, 2 * SLAB * s:2 * SLAB * s + 2]
                    else:
                        nc.vector.tensor_add(us[:, ssl], u, cb[:, csl])
                        rel = us.bitcast(BF16)[0:1, 2 * SLAB * s:2 * SLAB * s + 2]
                    slab_release.append(rel)
                    si += 1
                nc.vector.tensor_reduce(mnf[:, hm:hm + 1], us, axis=AX.X,
                                        op=ALU.min)
                nc.vector.tensor_scalar_mul(bias8[:, hm:hm + 1],
                                            mnf[:, hm:hm + 1], SCL)
                # exp(SCL*min - SCL*us) with row-sum accumulated into outpack
                nc.scalar.activation(
                    out=e2k, in_=us, func=AF.Exp, bias=bias8[:, hm:hm + 1],
                    scale=-SCL, accum_out=es8[:, hm:hm + 1])
                if h == 1:
                    # positive logits live at local cols 2048+128m+p
                    s512 = scr.tile([P, RB], F32, name=f"s512{m}", tag="s512",
                                    bufs=2)
                    nc.gpsimd.tensor_mul(s512, us[:, 0:RB],
                                         oneh[:, RB * m:RB * (m + 1)])
                    nc.scalar.activation(
                        out=junk, in_=s512, func=AF.Identity,
                        accum_out=upos4[:, m:m + 1])
        nc.sync.dma_start(out=out_d[:, 0:8], in_=bias8)
        nc.sync.dma_start(out=out_d[:, 8:16], in_=es8)
        nc.sync.dma_start(out=out_d[:, 16:20], in_=upos4)

    return nc


_NC_CACHE = None


def _get_nc():
    global _NC_CACHE
    if _NC_CACHE is None:
        nc = _build_nc()
        nc.finalize()  # runs Bacc.compile(): wait legalization for TRN2
        _NC_CACHE = nc
    return _NC_CACHE


def _host_planes(mu_x, sigma_x, mu_p, sigma_p):
    mus = np.concatenate([np.asarray(mu_x, np.float64),
                          np.asarray(mu_p, np.float64)], 0)
    sigmas = np.concatenate([np.asarray(sigma_x, np.float64),
                             np.asarray(sigma_p, np.float64)], 0)
    var = sigmas * sigmas
    inv = 1.0 / var
    planes = {
        "invT": inv.T,
        "qT": (var + mus * mus).T,
        "miT": (mus * inv).T,
        "mudT": (-2.0 * mus).T,
    }
    import ml_dtypes
    planes = {k: np.ascontiguousarray(v.astype(np.float32),
                                      ml_dtypes.bfloat16)
              for k, v in planes.items()}
    c = (mus * mus * inv).sum(1).astype(np.float32)
    return planes, c


def run_sharded(mu_x, sigma_x, mu_p, sigma_p, trace=False):
    planes, c = _host_planes(mu_x, sigma_x, mu_p, sigma_p)
    in_maps = []
    for k in range(NC):
        m = {name: np.ascontiguousarray(np.roll(v, -RB * k, axis=1))
             for name, v in planes.items()}
        ck = np.roll(c, -RB * k)
        m["cbB"] = np.ascontiguousarray(
            np.broadcast_to(ck[None, :], (P, NB)))
        m["cbKB"] = np.ascontiguousarray(
            np.broadcast_to((ck / P)[None, :], (P, NB)))
        in_maps.append(m)
    kwargs = {}
    if trace:
        kwargs = dict(trace=True, trace_cores=[0])
    br = run_bass_kernel_spmd(_get_nc(), in_maps, core_ids=list(range(NC)),
                              **kwargs)
    outs = np.stack([np.asarray(r["out"], np.float64) for r in br.results])
    smn = outs[:, :, 0:8]    # per-(h,m) SCL*min of us
    es = outs[:, :, 8:16]    # per-(h,m) sum exp(SCL*min - SCL*us)
    up = outs[:, :, 16:20]   # us at the positive column
    L = -smn + np.log(es)    # per-half log sum exp(-SCL*us)
    LSE = np.logaddexp(L[:, :, 0:4], L[:, :, 4:8])
    rl = LSE + SCL * up      # row loss (row consts cancel)
    n_classes = NB - 1
    to_mult = (n_classes - 1.0 / WEIGHT) / (n_classes - 1)
    to_add = -np.log(np.float32(to_mult))
    loss = np.float32(rl.sum() / NB - to_add)
    return loss, br


def kernel(z_hat, mu_x, sigma_x, mu_p, sigma_p):
    loss, _ = run_sharded(mu_x, sigma_x, mu_p, sigma_p)
    return np.asarray(loss, np.float32)
